# revision 47
# baseline (speedup 1.0000x reference)
"""BNext block (attention + FFN_1x1, binarized convs, frozen BN) on 8 TRN2 cores.

Data-parallel over batch (16 -> 2 images per core). Per core:
  - channels on partitions (2 c-tiles of 128), pixels (b, h, w) on the free dim
  - hardsign z computed on DVE as (x >= thr) - 0.5 into a zero-padded fp8
    buffer [128, kt, B, 58, 58] (the 2x is folded into the fp8 weight scales);
    3x3 binary conv = 9 shifted fp8 DoubleRow matmuls (each contracting both
    128-channel k-tiles at 0.5 cycles/col) per mtile accumulated in PSUM;
    weights are fp8 with per-output-channel pow2 scales folded into the bn1
    drain scale; bn1+prelu fused into the PSUM drain via ScalarE Prelu
  - SE means are linear: mean(mix) = s*mean(x) + (1-s)*mean(y); the s/(1-s)
    factors are folded into the SE w1 weights on the host; x sums are computed
    on the Pool engine, y1/u1 sums via DVE reduces over the bf16 tiles
  - residual/bn2 algebra collapses to outa = residual*(inv2*gate*y1 + 1)
    (bias2 folded into the z2 threshold / final bias / sum corrections);
    residual prelu = max(x, alpha*x) on DVE (alpha in [0,1])
  - elementwise work split across ACT/DVE/Pool; bf16 intermediates give DVE
    its fast modes; per-image pipelining: image 0's SE gate + postprocessing
    overlap image 1's conv matmuls; SE matmuls are placed in the PE stream
    where their inputs are already available
"""

import numpy as np

EPS = 1e-5
NCORES = 8
B, C, H, W = 16, 256, 56, 56
BP = B // NCORES            # images per core
HW = H * W                  # 3136
PIX = BP * HW               # 6272
CT = C // 128               # 2 c-tiles
HP, WP = H + 2, W + 2       # padded z: 58 x 58
RS = 8                      # conv chunk rows  -> N = 448
NCH = H // RS               # 7 conv chunks per image
SR2 = 28                    # a1/a4/f3 chunk rows (1568 elems)
NSC2 = H // SR2             # 2 per image
NV = 20

_CACHE = {}


def _build_program(loop_R=None, phase_limit=99, bench_mode=False, a1_mode="full"):
    import concourse.bass as bass
    import concourse.bacc as bacc
    import concourse.tile as tile
    from concourse import mybir

    AF = mybir.ActivationFunctionType
    ALU = mybir.AluOpType
    AX = mybir.AxisListType
    F32 = mybir.dt.float32
    F8 = mybir.dt.float8e4
    BF16 = mybir.dt.bfloat16
    DR = mybir.MatmulPerfMode.DoubleRow

    nc = bacc.Bacc("TRN2", target_bir_lowering=False, debug=False)

    KIN = "Internal" if bench_mode else "ExternalInput"
    KOUT = "Internal" if bench_mode else "ExternalOutput"
    xin = nc.dram_tensor("xin", [BP, C, H, W], BF16, kind=KIN).ap()
    wa = nc.dram_tensor("wa", [9, 128, CT, C], F8, kind=KIN).ap()
    wf = nc.dram_tensor("wf", [128, CT, C], F8, kind=KIN).ap()
    vecs_d = nc.dram_tensor("vecs", [CT, 128, NV], F32, kind=KIN).ap()
    b1a_d = nc.dram_tensor("b1a", [32, 1], F32, kind=KIN).ap()
    b1f_d = nc.dram_tensor("b1f", [32, 1], F32, kind=KIN).ap()
    # host folds s / (1-s) (and 1/HW) into the first SE matmul weights
    w1a_d = nc.dram_tensor("w1a", [2, CT, 128, 32], F32, kind=KIN).ap()
    w2a_d = nc.dram_tensor("w2a", [32, C], F32, kind=KIN).ap()
    w1f_d = nc.dram_tensor("w1f", [2, CT, 128, 32], F32, kind=KIN).ap()
    w2f_d = nc.dram_tensor("w2f", [32, C], F32, kind=KIN).ap()
    out_d = nc.dram_tensor("out", [BP, C, H, W], BF16, kind=KOUT).ap()
    tick_d = (nc.dram_tensor("tick", [1, 8], F32, kind="ExternalOutput").ap()
              if bench_mode else None)

    x_v = xin.rearrange("b (ct p) h w -> ct p b (h w)", ct=CT)
    out_v = out_d.rearrange("b (ct p) h w -> ct p b (h w)", ct=CT)

    (V_THR1, V_AL1, V_INV1, V_BIAS1, V_AL2, V_S, V_1MS, V_INV2, V_THR2,
     V_FINV1, V_FBIAS1, V_FAL2, V_FS, V_1MFS, V_FINV2, V_CFIN, V_B2A,
     V_B2F, V_SB2HW, V_BIAS2) = range(NV)

    with tile.TileContext(nc) as tc:
        import contextlib
        es = contextlib.ExitStack()
        with es:
            consts = es.enter_context(tc.tile_pool(name="consts", bufs=1))
            big = es.enter_context(tc.tile_pool(name="big", bufs=1))
            stream = es.enter_context(tc.tile_pool(name="stream", bufs=2))
            psum = es.enter_context(tc.tile_pool(name="psum", bufs=4, space="PSUM"))
            psum_se = es.enter_context(
                tc.tile_pool(name="psum_se", bufs=1, space="PSUM"))

            # ---- constants in ----
            vecs = [consts.tile([128, NV], F32, name=f"vecs{ct}") for ct in range(CT)]
            for ct in range(CT):
                nc.sync.dma_start(vecs[ct][:], vecs_d[ct])
            b1a = consts.tile([32, 1], F32)
            b1f = consts.tile([32, 1], F32)
            nc.sync.dma_start(b1a[:], b1a_d)
            nc.sync.dma_start(b1f[:], b1f_d)
            w1a = [[consts.tile([128, 32], F32, name=f"w1a{k}_{ct}")
                    for ct in range(CT)] for k in range(2)]
            w1f = [[consts.tile([128, 32], F32, name=f"w1f{k}_{ct}")
                    for ct in range(CT)] for k in range(2)]
            for k in range(2):
                for ct in range(CT):
                    nc.sync.dma_start(w1a[k][ct][:], w1a_d[k, ct])
                    nc.sync.dma_start(w1f[k][ct][:], w1f_d[k, ct])
            w2a = consts.tile([32, C], F32)
            w2f = consts.tile([32, C], F32)
            nc.sync.dma_start(w2a[:], w2a_d)
            nc.sync.dma_start(w2f[:], w2f_d)
            wconv = [consts.tile([128, CT, C], F8, name=f"wc{t}")
                     for t in range(9)]
            for t in range(9):
                nc.sync.dma_start(wconv[t][:], wa[t])
            wffn = consts.tile([128, CT, C], F8, name="wff")
            nc.sync.dma_start(wffn[:], wf)

            # ---- persistent buffers ----
            # single fp8 z buffer; dim1 = k-tile so DoubleRow matmuls can
            # contract both 128-channel halves in one instruction
            zpad = big.tile([128, CT, BP, HP, WP], F8, name="zpad", tag="zpad")
            z2v = zpad[:].rearrange("p c b h w -> p c (b h w)")
            # y1 / u1 share per-(ct,b) slots, bf16
            y1 = [[big.tile([128, HW], BF16, name=f"y1_{ct}_{b}",
                            tag=f"s1_{ct}_{b}")
                   for b in range(BP)] for ct in range(CT)]
            outa = [big.tile([128, PIX], BF16, name=f"outa{ct}", tag=f"oa{ct}")
                    for ct in range(CT)]

            sums2 = [consts.tile([128, 16], F32, name=f"sums2_{ct}")
                     for ct in range(CT)]
            # sxr cols: setype*4 + b*2 + {0: pooled-in0, 1: pooled-in1}
            sxr = [consts.tile([128, 8], F32, name=f"sxr{ct}") for ct in range(CT)]
            scr = consts.tile([1, 2], F32, name="scr")

            ps_x = [consts.tile([128, 8], F32, name=f"ps_x_{ct}")
                    for ct in range(CT)]
            ps_oa = [consts.tile([128, 8], F32, name=f"ps_oa_{ct}")
                     for ct in range(CT)]
            ps_y1 = [consts.tile([128, 16], F32, name=f"ps_y1_{ct}")
                     for ct in range(CT)]
            ps_u = [consts.tile([128, 16], F32, name=f"ps_u_{ct}")
                    for ct in range(CT)]


            # pin the activation table set (sigmoid_and_others has every
            # function this kernel uses) before any other ACT work
            nc.vector.memset(scr[:], 0.0)
            nc.scalar.activation(scr[:, 0:1], scr[:, 0:1], AF.Sigmoid,
                                 bias=0.0, scale=1.0)

            # zpad borders
            zp = zpad[:]
            nc.vector.memset(zp[:, :, :, 0, :], 0.0)
            nc.vector.memset(zp[:, :, :, HP - 1, :], 0.0)
            nc.vector.memset(zp[:, :, :, :, 0], 0.0)
            nc.vector.memset(zp[:, :, :, :, WP - 1], 0.0)

            # bench loop: consts/borders stay hoisted above (loaded once)
            if loop_R is not None:
                es.enter_context(tc.For_i(0, loop_R, 1))
            if bench_mode:
                tick_sb = consts.tile([1, 8], F32, name="tick_sb")
                nc.vector.memset(tick_sb[:], 1.0)
                nc.sync.dma_start(tick_d, tick_sb[:])

            def sign_to(eng, dst, src, thr_ap):
                # dst = (src >= thr) - 0.5  in {-0.5, +0.5}
                eng.tensor_scalar(dst, src, thr_ap, 0.5,
                                  op0=ALU.is_ge, op1=ALU.subtract)

            # ===== helpers =====
            xq = [nc.gpsimd, nc.sync, nc.scalar]

            def phase_a1(b):
                for s in range(NSC2):
                    for ct in range(CT):
                        xt = stream.tile([128, SR2 * W], BF16, tag="xs", bufs=8)
                        nc.gpsimd.dma_start(
                            xt[:], x_v[ct][:, b, s * SR2 * W:(s + 1) * SR2 * W])
                        if a1_mode == "dma":
                            continue
                        zdst = zpad[:, ct, b,
                                    1 + s * SR2:1 + (s + 1) * SR2, 1:1 + W]
                        sign_to(nc.vector,
                                zdst, xt[:].rearrange("p (r w) -> p r w", w=W),
                                vecs[ct][:, V_THR1:V_THR1 + 1])
                        if a1_mode == "dma_sign":
                            continue
                        seg = slice(b * HW + s * SR2 * W, b * HW + (s + 1) * SR2 * W)
                        # residual = max(x, alpha*x)   (alpha in [0,1])
                        nc.vector.scalar_tensor_tensor(
                            outa[ct][:, seg], xt[:],
                            vecs[ct][:, V_AL1:V_AL1 + 1], xt[:],
                            op0=ALU.mult, op1=ALU.max)
                        if a1_mode == "dma_sign_prelu":
                            continue
                        col = b * NSC2 + s
                        nc.vector.tensor_reduce(
                            ps_x[ct][:, col:col + 1], xt[:],
                            axis=AX.X, op=ALU.add)

            def phase_conv(b, j0=0, j1=NCH):
                for j in range(j0, j1):
                    for mt in range(CT):
                        pt = psum.tile([128, RS, W], F32, tag="conv")
                        for dy in range(3):
                            for dx in range(3):
                                t = dy * 3 + dx
                                rhs = zpad[:, :, b,
                                           j * RS + dy:j * RS + dy + RS,
                                           dx:dx + W]
                                nc.tensor.matmul(
                                    pt[:],
                                    wconv[t][:, :, mt * 128:(mt + 1) * 128],
                                    rhs, start=(t == 0), stop=(t == 8),
                                    perf_mode=DR)
                        col = b * NCH + j
                        ydst = y1[mt][b][:, j * RS * W:(j + 1) * RS * W]
                        nc.scalar.activation(
                            ydst.rearrange("p (r w) -> p r w", w=W), pt[:],
                            AF.Prelu,
                            bias=vecs[mt][:, V_BIAS1:V_BIAS1 + 1],
                            scale=vecs[mt][:, V_INV1:V_INV1 + 1],
                            alpha=vecs[mt][:, V_AL2:V_AL2 + 1],
                            accum_out=ps_y1[mt][:, col:col + 1])

            def se_gate(b, setype, w1k, w2, b1t, vb2, vpost, gcol,
                        sum_corr_col=None):
                """SE gate for image b: gate[gcol+b] = post * sigmoid(...)"""
                base = setype * 4 + b * 2
                for ct in range(CT):
                    if setype == 0:
                        nc.vector.tensor_reduce(
                            sxr[ct][:, base:base + 1],
                            ps_x[ct][:, b * NSC2:(b + 1) * NSC2],
                            axis=AX.X, op=ALU.add)
                        nc.vector.tensor_reduce(
                            sxr[ct][:, base + 1:base + 2],
                            ps_y1[ct][:, b * NCH:(b + 1) * NCH],
                            axis=AX.X, op=ALU.add)
                    else:
                        nc.vector.tensor_reduce(
                            sxr[ct][:, base:base + 1],
                            ps_oa[ct][:, b * NSC2:(b + 1) * NSC2],
                            axis=AX.X, op=ALU.add)
                        nc.vector.tensor_scalar(
                            sxr[ct][:, base:base + 1], sxr[ct][:, base:base + 1],
                            vecs[ct][:, sum_corr_col:sum_corr_col + 1], None,
                            op0=ALU.add)
                        nc.vector.tensor_reduce(
                            sxr[ct][:, base + 1:base + 2],
                            ps_u[ct][:, b * NCH:(b + 1) * NCH],
                            axis=AX.X, op=ALU.add)
                hp = psum_se.tile([32, 1], F32, tag="seh")
                first = True
                for k in range(2):
                    for ct in range(CT):
                        nc.tensor.matmul(hp[:], w1k[k][ct][:],
                                         sxr[ct][:, base + k:base + k + 1],
                                         start=first,
                                         stop=(k == 1 and ct == CT - 1))
                        first = False
                hs = consts.tile([32, 1], F32, tag="hs")
                nc.scalar.activation(hs[:], hp[:], AF.Relu, bias=b1t[:], scale=1.0)
                for mt in range(CT):
                    gp = psum_se.tile([128, 1], F32, tag="seg")
                    nc.tensor.matmul(gp[:], w2[:, mt * 128:(mt + 1) * 128], hs[:],
                                     start=True, stop=True)
                    nc.scalar.activation(
                        sums2[mt][:, gcol + b:gcol + b + 1], gp[:], AF.Sigmoid,
                        bias=vecs[mt][:, vb2:vb2 + 1], scale=1.0)
                    nc.vector.tensor_scalar(
                        sums2[mt][:, gcol + b:gcol + b + 1],
                        sums2[mt][:, gcol + b:gcol + b + 1],
                        vecs[mt][:, vpost:vpost + 1], None, op0=ALU.mult)

            def phase_a4(b):
                for ct in range(CT):
                    for s in range(NSC2):
                        seg = slice(b * HW + s * SR2 * W,
                                    b * HW + (s + 1) * SR2 * W)
                        yseg = slice(s * SR2 * W, (s + 1) * SR2 * W)
                        t = stream.tile([128, SR2 * W], BF16, tag="work", bufs=6)
                        if s == 0:
                            nc.vector.tensor_scalar(
                                t[:], y1[ct][b][:, yseg],
                                sums2[ct][:, 8 + b:9 + b], 1.0,
                                op0=ALU.mult, op1=ALU.add)
                        else:
                            nc.scalar.activation(
                                t[:], y1[ct][b][:, yseg], AF.Identity,
                                bias=1.0, scale=sums2[ct][:, 8 + b:9 + b])
                        col = b * NSC2 + s
                        nc.vector.scalar_tensor_tensor(
                            outa[ct][:, seg], t[:], 0.0, outa[ct][:, seg],
                            op0=ALU.bypass, op1=ALU.mult,
                            accum_out=ps_oa[ct][:, col:col + 1])
                        z2dst = z2v[:, ct,
                                    b * HW + s * SR2 * W:b * HW + (s + 1) * SR2 * W]
                        sign_to(nc.vector, z2dst, outa[ct][:, seg],
                                vecs[ct][:, V_THR2:V_THR2 + 1])

            def phase_f1(b):
                for j in range(NCH):
                    seg = slice(b * HW + j * RS * W, b * HW + (j + 1) * RS * W)
                    for mt in range(CT):
                        pt = psum.tile([128, RS * W], F32, tag="ffn", bufs=2)
                        nc.tensor.matmul(
                            pt[:], wffn[:, :, mt * 128:(mt + 1) * 128],
                            z2v[:, :, seg], start=True, stop=True,
                            perf_mode=DR)
                        col = b * NCH + j
                        ydst = y1[mt][b][:, j * RS * W:(j + 1) * RS * W]
                        nc.scalar.activation(
                            ydst, pt[:], AF.Prelu,
                            bias=vecs[mt][:, V_FBIAS1:V_FBIAS1 + 1],
                            scale=vecs[mt][:, V_FINV1:V_FINV1 + 1],
                            alpha=vecs[mt][:, V_FAL2:V_FAL2 + 1],
                            accum_out=ps_u[mt][:, col:col + 1])

            def phase_f3(b):
                for ct in range(CT):
                    for s in range(NSC2):
                        seg = slice(b * HW + s * SR2 * W,
                                    b * HW + (s + 1) * SR2 * W)
                        yseg = slice(s * SR2 * W, (s + 1) * SR2 * W)
                        t = stream.tile([128, SR2 * W], BF16, tag="work", bufs=6)
                        if s == 0:
                            nc.vector.tensor_scalar(
                                t[:], y1[ct][b][:, yseg],
                                sums2[ct][:, 10 + b:11 + b],
                                vecs[ct][:, V_CFIN:V_CFIN + 1],
                                op0=ALU.mult, op1=ALU.add)
                        else:
                            nc.scalar.activation(
                                t[:], y1[ct][b][:, yseg], AF.Identity,
                                bias=vecs[ct][:, V_CFIN:V_CFIN + 1],
                                scale=sums2[ct][:, 10 + b:11 + b])
                        fin = stream.tile([128, SR2 * W], BF16, tag="fin", bufs=4)
                        nc.vector.tensor_add(fin[:], t[:], outa[ct][:, seg])
                        st_eng = nc.sync if s == 0 else nc.gpsimd
                        st_eng.dma_start(
                            out_v[ct][:, b, s * SR2 * W:(s + 1) * SR2 * W],
                            fin[:])

            # ===== schedule =====
            # b0's gate + postprocessing are emitted BETWEEN conv(0) and
            # conv(1) so they sit ahead of conv-b1 in the per-engine FIFOs
            # and execute during conv-b1's matmuls.
            if phase_limit >= 1:
                phase_a1(0)
                phase_a1(1)
            if phase_limit >= 2:
                phase_conv(0)
                phase_conv(1, 0, 1)
            if phase_limit >= 3:
                se_gate(0, 0, w1a, w2a, b1a, V_B2A, V_INV2, 8)
            if phase_limit >= 2:
                phase_conv(1, 1, NCH)
            if phase_limit >= 4:
                phase_a4(0)
            if phase_limit >= 3:
                se_gate(1, 0, w1a, w2a, b1a, V_B2A, V_INV2, 8)
            if phase_limit >= 5:
                phase_f1(0)
            if phase_limit >= 6:
                se_gate(0, 1, w1f, w2f, b1f, V_B2F, V_FINV2, 10,
                        sum_corr_col=V_SB2HW)
            if phase_limit >= 4:
                phase_a4(1)
            if phase_limit >= 7:
                phase_f3(0)
            if phase_limit >= 5:
                phase_f1(1)
            if phase_limit >= 6:
                se_gate(1, 1, w1f, w2f, b1f, V_B2F, V_FINV2, 10,
                        sum_corr_col=V_SB2HW)
            if phase_limit >= 7:
                phase_f3(1)

    nc.compile()
    return nc


def _host_prep(inputs):
    import ml_dtypes
    f8 = ml_dtypes.float8_e4m3
    f32 = np.float32
    g1, be1, m1, v1 = (inputs["a_bn1"][i].astype(f32) for i in range(4))
    g2, be2, m2, v2 = (inputs["a_bn2"][i].astype(f32) for i in range(4))
    fg1, fbe1, fm1, fv1 = (inputs["f_bn1"][i].astype(f32) for i in range(4))
    fg2, fbe2, fm2, fv2 = (inputs["f_bn2"][i].astype(f32) for i in range(4))
    inv1 = g1 / np.sqrt(v1 + EPS)
    bias1 = be1 - m1 * inv1
    inv2 = g2 / np.sqrt(v2 + EPS)
    bias2 = be2 - m2 * inv2
    finv1 = fg1 / np.sqrt(fv1 + EPS)
    fbias1 = fbe1 - fm1 * finv1
    finv2 = fg2 / np.sqrt(fv2 + EPS)
    fbias2 = fbe2 - fm2 * finv2

    s = inputs["a_scale"].astype(f32)
    fs = inputs["f_scale"].astype(f32)

    # fp8 per-output-channel pow2 scaling (folded into the bn1 drain scale);
    # z is stored as +-0.5 so the drain scale carries an extra 2x
    bw = np.clip(inputs["a_w"].astype(f32), -1.0, 1.0)        # [O, I, 3, 3]
    mxa = np.abs(bw.reshape(C, -1)).max(axis=1)
    sa = np.exp2(np.floor(np.log2(224.0 / np.where(mxa == 0, 1.0, mxa))))
    bw2 = np.clip(inputs["f_w"].astype(f32), -1.0, 1.0)       # [O, I]
    mxf = np.abs(bw2).max(axis=1)
    sf = np.exp2(np.floor(np.log2(224.0 / np.where(mxf == 0, 1.0, mxf))))

    vecs = np.zeros((C, NV), f32)
    vecs[:, V_THR1] = -inputs["a_move"]
    vecs[:, 1] = inputs["a_alpha1"]
    vecs[:, 2] = 2.0 * inv1 / sa
    vecs[:, 3] = bias1
    vecs[:, 4] = inputs["a_alpha2"]
    vecs[:, 5] = s
    vecs[:, 6] = 1.0 - s
    vecs[:, 7] = inv2
    vecs[:, V_THR2] = -(bias2 + inputs["f_move"])
    vecs[:, 9] = 2.0 * finv1 / sf
    vecs[:, 10] = fbias1
    vecs[:, 11] = inputs["f_alpha2"]
    vecs[:, 12] = fs
    vecs[:, 13] = 1.0 - fs
    vecs[:, 14] = finv2
    vecs[:, 15] = fbias2 + bias2
    vecs[:, 16] = inputs["a_se_b2"]
    vecs[:, 17] = inputs["f_se_b2"]
    vecs[:, 18] = float(HW) * bias2
    vecs[:, 19] = bias2
    vecs_ct = np.ascontiguousarray(vecs.reshape(CT, 128, NV))

    # wa[t, p, kt, o] = bw[o, kt*128+p, ky, kx] * sa[o]
    bws = bw * sa[:, None, None, None]
    bwT = np.transpose(bws, (1, 0, 2, 3)).reshape(CT, 128, C, 3, 3)
    wa_h = np.zeros((9, 128, CT, C), f32)
    for ky in range(3):
        for kx in range(3):
            wa_h[ky * 3 + kx] = np.transpose(bwT[:, :, :, ky, kx], (1, 0, 2))
    wa_h = wa_h.astype(f8)
    bw2s = (bw2 * sf[:, None]).T.reshape(CT, 128, C)
    wfm = np.ascontiguousarray(np.transpose(bw2s, (1, 0, 2))).astype(f8)

    def w1_fold(w1, sv):
        w1t = w1.astype(f32).T / float(HW)          # [256, 32]
        out = np.zeros((2, CT, 128, 32), f32)
        out[0] = (w1t * sv[:, None]).reshape(CT, 128, 32)
        out[1] = (w1t * (1.0 - sv)[:, None]).reshape(CT, 128, 32)
        return out

    common = {
        "wa": wa_h, "wf": wfm, "vecs": vecs_ct,
        "b1a": inputs["a_se_b1"].astype(f32).reshape(32, 1),
        "b1f": inputs["f_se_b1"].astype(f32).reshape(32, 1),
        "w1a": w1_fold(inputs["a_se_w1"], s),
        "w2a": np.ascontiguousarray(inputs["a_se_w2"].astype(f32).T),
        "w1f": w1_fold(inputs["f_se_w1"], fs),
        "w2f": np.ascontiguousarray(inputs["f_se_w2"].astype(f32).T),
    }
    return common


V_THR1 = 0
V_THR2 = 8


def kernel(**inputs):
    from concourse import bass_utils

    if "nc" not in _CACHE:
        _CACHE["nc"] = _build_program()
    nc = _CACHE["nc"]

    import ml_dtypes
    common = _host_prep(inputs)
    x = np.ascontiguousarray(
        inputs["x"].astype(np.float32).astype(ml_dtypes.bfloat16))
    in_maps = []
    for c in range(NCORES):
        m = dict(common)
        m["xin"] = np.ascontiguousarray(x[c * BP:(c + 1) * BP])
        in_maps.append(m)

    res = None
    for attempt in range(3):
        try:
            res = bass_utils.run_bass_kernel_spmd(
                nc, in_maps, core_ids=list(range(NCORES)))
            break
        except Exception:
            # transient device wedge on a freshly loaded NEFF: retry
            if attempt == 2:
                raise
    out = np.empty((B, C, H, W), np.float32)
    for c in range(NCORES):
        out[c * BP:(c + 1) * BP] = res.results[c]["out"].astype(np.float32)
    return out


# revision 48
# speedup vs baseline: 1.3560x; 1.3560x over previous
"""BNext block (attention + FFN_1x1, binarized convs, frozen BN) on 8 TRN2 cores.

Data-parallel over batch (16 -> 2 images per core). Per core:
  - channels on partitions (2 c-tiles of 128), pixels (b, h, w) on the free dim
  - hardsign z computed on DVE as (x >= thr) - 0.5 into a zero-padded fp8
    buffer [128, kt, B, 58, 58] (the 2x is folded into the fp8 weight scales);
    3x3 binary conv = 9 shifted fp8 DoubleRow matmuls (each contracting both
    128-channel k-tiles at 0.5 cycles/col) per mtile accumulated in PSUM;
    weights are fp8 with per-output-channel pow2 scales folded into the bn1
    drain scale; bn1+prelu fused into the PSUM drain via ScalarE Prelu
  - SE means are linear: mean(mix) = s*mean(x) + (1-s)*mean(y); the s/(1-s)
    factors are folded into the SE w1 weights on the host; x sums are computed
    on the Pool engine, y1/u1 sums via DVE reduces over the bf16 tiles
  - residual/bn2 algebra collapses to outa = residual*(inv2*gate*y1 + 1)
    (bias2 folded into the z2 threshold / final bias / sum corrections);
    residual prelu = max(x, alpha*x) on DVE (alpha in [0,1])
  - elementwise work split across ACT/DVE/Pool; bf16 intermediates give DVE
    its fast modes; per-image pipelining: image 0's SE gate + postprocessing
    overlap image 1's conv matmuls; SE matmuls are placed in the PE stream
    where their inputs are already available
"""

import numpy as np

EPS = 1e-5
NCORES = 8
B, C, H, W = 16, 256, 56, 56
BP = B // NCORES            # images per core
HW = H * W                  # 3136
PIX = BP * HW               # 6272
CT = C // 128               # 2 c-tiles
HP, WP = H + 2, W + 2       # padded z: 58 x 58
RS = 8                      # conv chunk rows  -> N = 448
NCH = H // RS               # 7 conv chunks per image
SR2 = 28                    # a1/a4/f3 chunk rows (1568 elems)
NSC2 = H // SR2             # 2 per image
NV = 20

_CACHE = {}


def _build_program(loop_R=None, phase_limit=99, bench_mode=False, a1_mode="full"):
    import concourse.bass as bass
    import concourse.bacc as bacc
    import concourse.tile as tile
    from concourse import mybir

    AF = mybir.ActivationFunctionType
    ALU = mybir.AluOpType
    AX = mybir.AxisListType
    F32 = mybir.dt.float32
    F8 = mybir.dt.float8e4
    BF16 = mybir.dt.bfloat16
    DR = mybir.MatmulPerfMode.DoubleRow

    nc = bacc.Bacc("TRN2", target_bir_lowering=False, debug=False)

    KIN = "Internal" if bench_mode else "ExternalInput"
    KOUT = "Internal" if bench_mode else "ExternalOutput"
    xin = nc.dram_tensor("xin", [BP, C, H, W], BF16, kind=KIN).ap()
    wa = nc.dram_tensor("wa", [9, 128, CT, C], F8, kind=KIN).ap()
    wf = nc.dram_tensor("wf", [128, CT, C], F8, kind=KIN).ap()
    vecs_d = nc.dram_tensor("vecs", [CT, 128, NV], F32, kind=KIN).ap()
    b1a_d = nc.dram_tensor("b1a", [32, 1], F32, kind=KIN).ap()
    b1f_d = nc.dram_tensor("b1f", [32, 1], F32, kind=KIN).ap()
    # host folds s / (1-s) (and 1/HW) into the first SE matmul weights
    w1a_d = nc.dram_tensor("w1a", [2, CT, 128, 32], F32, kind=KIN).ap()
    w2a_d = nc.dram_tensor("w2a", [32, C], F32, kind=KIN).ap()
    w1f_d = nc.dram_tensor("w1f", [2, CT, 128, 32], F32, kind=KIN).ap()
    w2f_d = nc.dram_tensor("w2f", [32, C], F32, kind=KIN).ap()
    out_d = nc.dram_tensor("out", [BP, C, H, W], BF16, kind=KOUT).ap()
    tick_d = (nc.dram_tensor("tick", [1, 8], F32, kind="ExternalOutput").ap()
              if bench_mode else None)

    x_v = xin.rearrange("b (ct p) h w -> ct p b (h w)", ct=CT)
    out_v = out_d.rearrange("b (ct p) h w -> ct p b (h w)", ct=CT)

    (V_THR1, V_AL1, V_INV1, V_BIAS1, V_AL2, V_S, V_1MS, V_INV2, V_THR2,
     V_FINV1, V_FBIAS1, V_FAL2, V_FS, V_1MFS, V_FINV2, V_CFIN, V_B2A,
     V_B2F, V_SB2HW, V_BIAS2) = range(NV)

    with tile.TileContext(nc) as tc:
        import contextlib
        es = contextlib.ExitStack()
        with es:
            consts = es.enter_context(tc.tile_pool(name="consts", bufs=1))
            big = es.enter_context(tc.tile_pool(name="big", bufs=1))
            stream = es.enter_context(tc.tile_pool(name="stream", bufs=2))
            psum = es.enter_context(tc.tile_pool(name="psum", bufs=4, space="PSUM"))
            psum_se = es.enter_context(
                tc.tile_pool(name="psum_se", bufs=1, space="PSUM"))

            # ---- constants in ----
            vecs = [consts.tile([128, NV], F32, name=f"vecs{ct}") for ct in range(CT)]
            for ct in range(CT):
                nc.sync.dma_start(vecs[ct][:], vecs_d[ct])
            b1a = consts.tile([32, 1], F32)
            b1f = consts.tile([32, 1], F32)
            nc.sync.dma_start(b1a[:], b1a_d)
            nc.sync.dma_start(b1f[:], b1f_d)
            w1a = [[consts.tile([128, 32], F32, name=f"w1a{k}_{ct}")
                    for ct in range(CT)] for k in range(2)]
            w1f = [[consts.tile([128, 32], F32, name=f"w1f{k}_{ct}")
                    for ct in range(CT)] for k in range(2)]
            for k in range(2):
                for ct in range(CT):
                    nc.sync.dma_start(w1a[k][ct][:], w1a_d[k, ct])
                    nc.sync.dma_start(w1f[k][ct][:], w1f_d[k, ct])
            w2a = consts.tile([32, C], F32)
            w2f = consts.tile([32, C], F32)
            nc.sync.dma_start(w2a[:], w2a_d)
            nc.sync.dma_start(w2f[:], w2f_d)
            wconv = [consts.tile([128, CT, C], F8, name=f"wc{t}")
                     for t in range(9)]
            for t in range(9):
                nc.sync.dma_start(wconv[t][:], wa[t])
            wffn = consts.tile([128, CT, C], F8, name="wff")
            nc.sync.dma_start(wffn[:], wf)

            # ---- persistent buffers ----
            # single fp8 z buffer; dim1 = k-tile so DoubleRow matmuls can
            # contract both 128-channel halves in one instruction
            zpad = big.tile([128, CT, BP, HP, WP], F8, name="zpad", tag="zpad")
            z2v = zpad[:].rearrange("p c b h w -> p c (b h w)")
            # y1 / u1 share per-(ct,b) slots, bf16
            y1 = [[big.tile([128, HW], BF16, name=f"y1_{ct}_{b}",
                            tag=f"s1_{ct}_{b}")
                   for b in range(BP)] for ct in range(CT)]
            outa = [big.tile([128, PIX], BF16, name=f"outa{ct}", tag=f"oa{ct}")
                    for ct in range(CT)]

            sums2 = [consts.tile([128, 16], F32, name=f"sums2_{ct}")
                     for ct in range(CT)]
            # sxr cols: setype*4 + b*2 + {0: pooled-in0, 1: pooled-in1}
            sxr = [consts.tile([128, 8], F32, name=f"sxr{ct}") for ct in range(CT)]
            scr = consts.tile([1, 2], F32, name="scr")

            ps_x = [consts.tile([128, 8], F32, name=f"ps_x_{ct}")
                    for ct in range(CT)]
            ps_oa = [consts.tile([128, 8], F32, name=f"ps_oa_{ct}")
                     for ct in range(CT)]
            ps_y1 = [consts.tile([128, 16], F32, name=f"ps_y1_{ct}")
                     for ct in range(CT)]
            ps_u = [consts.tile([128, 16], F32, name=f"ps_u_{ct}")
                    for ct in range(CT)]


            # pin the activation table set (sigmoid_and_others has every
            # function this kernel uses) before any other ACT work
            nc.vector.memset(scr[:], 0.0)
            nc.scalar.activation(scr[:, 0:1], scr[:, 0:1], AF.Sigmoid,
                                 bias=0.0, scale=1.0)

            # zpad borders
            zp = zpad[:]
            nc.vector.memset(zp[:, :, :, 0, :], 0.0)
            nc.vector.memset(zp[:, :, :, HP - 1, :], 0.0)
            nc.vector.memset(zp[:, :, :, :, 0], 0.0)
            nc.vector.memset(zp[:, :, :, :, WP - 1], 0.0)

            # bench loop: consts/borders stay hoisted above (loaded once)
            if loop_R is not None:
                es.enter_context(tc.For_i(0, loop_R, 1))
            if bench_mode:
                tick_sb = consts.tile([1, 8], F32, name="tick_sb")
                nc.vector.memset(tick_sb[:], 1.0)
                nc.sync.dma_start(tick_d, tick_sb[:])

            def sign_to(eng, dst, src, thr_ap):
                # dst = (src >= thr) - 0.5  in {-0.5, +0.5}
                eng.tensor_scalar(dst, src, thr_ap, 0.5,
                                  op0=ALU.is_ge, op1=ALU.subtract)

            # ===== helpers =====
            xq = [nc.gpsimd, nc.sync, nc.scalar]

            def phase_a1(b):
                for s in range(NSC2):
                    for ct in range(CT):
                        xt = stream.tile([128, SR2 * W], BF16, tag="xs", bufs=8)
                        xq[(b * 4 + s * CT + ct) % 3].dma_start(
                            xt[:], x_v[ct][:, b, s * SR2 * W:(s + 1) * SR2 * W])
                        if a1_mode == "dma":
                            continue
                        zdst = zpad[:, ct, b,
                                    1 + s * SR2:1 + (s + 1) * SR2, 1:1 + W]
                        sign_to(nc.vector,
                                zdst, xt[:].rearrange("p (r w) -> p r w", w=W),
                                vecs[ct][:, V_THR1:V_THR1 + 1])
                        if a1_mode == "dma_sign":
                            continue
                        seg = slice(b * HW + s * SR2 * W, b * HW + (s + 1) * SR2 * W)
                        # residual = max(x, alpha*x)   (alpha in [0,1])
                        nc.vector.scalar_tensor_tensor(
                            outa[ct][:, seg], xt[:],
                            vecs[ct][:, V_AL1:V_AL1 + 1], xt[:],
                            op0=ALU.mult, op1=ALU.max)
                        if a1_mode == "dma_sign_prelu":
                            continue
                        col = b * NSC2 + s
                        nc.vector.tensor_reduce(
                            ps_x[ct][:, col:col + 1], xt[:],
                            axis=AX.X, op=ALU.add)

            def phase_conv(b, j0=0, j1=NCH):
                for j in range(j0, j1):
                    for mt in range(CT):
                        pt = psum.tile([128, RS, W], F32, tag="conv")
                        for dy in range(3):
                            for dx in range(3):
                                t = dy * 3 + dx
                                rhs = zpad[:, :, b,
                                           j * RS + dy:j * RS + dy + RS,
                                           dx:dx + W]
                                nc.tensor.matmul(
                                    pt[:],
                                    wconv[t][:, :, mt * 128:(mt + 1) * 128],
                                    rhs, start=(t == 0), stop=(t == 8),
                                    perf_mode=DR)
                        col = b * NCH + j
                        ydst = y1[mt][b][:, j * RS * W:(j + 1) * RS * W]
                        nc.scalar.activation(
                            ydst.rearrange("p (r w) -> p r w", w=W), pt[:],
                            AF.Prelu,
                            bias=vecs[mt][:, V_BIAS1:V_BIAS1 + 1],
                            scale=vecs[mt][:, V_INV1:V_INV1 + 1],
                            alpha=vecs[mt][:, V_AL2:V_AL2 + 1],
                            accum_out=ps_y1[mt][:, col:col + 1])

            def se_gate(b, setype, w1k, w2, b1t, vb2, vpost, gcol,
                        sum_corr_col=None):
                """SE gate for image b: gate[gcol+b] = post * sigmoid(...)"""
                base = setype * 4 + b * 2
                for ct in range(CT):
                    if setype == 0:
                        nc.vector.tensor_reduce(
                            sxr[ct][:, base:base + 1],
                            ps_x[ct][:, b * NSC2:(b + 1) * NSC2],
                            axis=AX.X, op=ALU.add)
                        nc.vector.tensor_reduce(
                            sxr[ct][:, base + 1:base + 2],
                            ps_y1[ct][:, b * NCH:(b + 1) * NCH],
                            axis=AX.X, op=ALU.add)
                    else:
                        nc.vector.tensor_reduce(
                            sxr[ct][:, base:base + 1],
                            ps_oa[ct][:, b * NSC2:(b + 1) * NSC2],
                            axis=AX.X, op=ALU.add)
                        nc.vector.tensor_scalar(
                            sxr[ct][:, base:base + 1], sxr[ct][:, base:base + 1],
                            vecs[ct][:, sum_corr_col:sum_corr_col + 1], None,
                            op0=ALU.add)
                        nc.vector.tensor_reduce(
                            sxr[ct][:, base + 1:base + 2],
                            ps_u[ct][:, b * NCH:(b + 1) * NCH],
                            axis=AX.X, op=ALU.add)
                hp = psum_se.tile([32, 1], F32, tag="seh")
                first = True
                for k in range(2):
                    for ct in range(CT):
                        nc.tensor.matmul(hp[:], w1k[k][ct][:],
                                         sxr[ct][:, base + k:base + k + 1],
                                         start=first,
                                         stop=(k == 1 and ct == CT - 1))
                        first = False
                hs = consts.tile([32, 1], F32, tag="hs")
                nc.scalar.activation(hs[:], hp[:], AF.Relu, bias=b1t[:], scale=1.0)
                for mt in range(CT):
                    gp = psum_se.tile([128, 1], F32, tag="seg")
                    nc.tensor.matmul(gp[:], w2[:, mt * 128:(mt + 1) * 128], hs[:],
                                     start=True, stop=True)
                    nc.scalar.activation(
                        sums2[mt][:, gcol + b:gcol + b + 1], gp[:], AF.Sigmoid,
                        bias=vecs[mt][:, vb2:vb2 + 1], scale=1.0)
                    nc.vector.tensor_scalar(
                        sums2[mt][:, gcol + b:gcol + b + 1],
                        sums2[mt][:, gcol + b:gcol + b + 1],
                        vecs[mt][:, vpost:vpost + 1], None, op0=ALU.mult)

            def phase_a4(b):
                for ct in range(CT):
                    for s in range(NSC2):
                        seg = slice(b * HW + s * SR2 * W,
                                    b * HW + (s + 1) * SR2 * W)
                        yseg = slice(s * SR2 * W, (s + 1) * SR2 * W)
                        t = stream.tile([128, SR2 * W], BF16, tag="work", bufs=6)
                        if s == 0:
                            nc.vector.tensor_scalar(
                                t[:], y1[ct][b][:, yseg],
                                sums2[ct][:, 8 + b:9 + b], 1.0,
                                op0=ALU.mult, op1=ALU.add)
                        else:
                            nc.scalar.activation(
                                t[:], y1[ct][b][:, yseg], AF.Identity,
                                bias=1.0, scale=sums2[ct][:, 8 + b:9 + b])
                        col = b * NSC2 + s
                        nc.vector.scalar_tensor_tensor(
                            outa[ct][:, seg], t[:], 0.0, outa[ct][:, seg],
                            op0=ALU.bypass, op1=ALU.mult,
                            accum_out=ps_oa[ct][:, col:col + 1])
                        z2dst = z2v[:, ct,
                                    b * HW + s * SR2 * W:b * HW + (s + 1) * SR2 * W]
                        sign_to(nc.vector, z2dst, outa[ct][:, seg],
                                vecs[ct][:, V_THR2:V_THR2 + 1])

            def phase_f1(b):
                for j in range(NCH):
                    seg = slice(b * HW + j * RS * W, b * HW + (j + 1) * RS * W)
                    for mt in range(CT):
                        pt = psum.tile([128, RS * W], F32, tag="ffn", bufs=2)
                        nc.tensor.matmul(
                            pt[:], wffn[:, :, mt * 128:(mt + 1) * 128],
                            z2v[:, :, seg], start=True, stop=True,
                            perf_mode=DR)
                        col = b * NCH + j
                        ydst = y1[mt][b][:, j * RS * W:(j + 1) * RS * W]
                        nc.scalar.activation(
                            ydst, pt[:], AF.Prelu,
                            bias=vecs[mt][:, V_FBIAS1:V_FBIAS1 + 1],
                            scale=vecs[mt][:, V_FINV1:V_FINV1 + 1],
                            alpha=vecs[mt][:, V_FAL2:V_FAL2 + 1],
                            accum_out=ps_u[mt][:, col:col + 1])

            def phase_f3(b):
                for ct in range(CT):
                    for s in range(NSC2):
                        seg = slice(b * HW + s * SR2 * W,
                                    b * HW + (s + 1) * SR2 * W)
                        yseg = slice(s * SR2 * W, (s + 1) * SR2 * W)
                        t = stream.tile([128, SR2 * W], BF16, tag="work", bufs=6)
                        if s == 0:
                            nc.vector.tensor_scalar(
                                t[:], y1[ct][b][:, yseg],
                                sums2[ct][:, 10 + b:11 + b],
                                vecs[ct][:, V_CFIN:V_CFIN + 1],
                                op0=ALU.mult, op1=ALU.add)
                        else:
                            nc.scalar.activation(
                                t[:], y1[ct][b][:, yseg], AF.Identity,
                                bias=vecs[ct][:, V_CFIN:V_CFIN + 1],
                                scale=sums2[ct][:, 10 + b:11 + b])
                        fin = stream.tile([128, SR2 * W], BF16, tag="fin", bufs=4)
                        nc.vector.tensor_add(fin[:], t[:], outa[ct][:, seg])
                        st_eng = nc.sync if s == 0 else nc.gpsimd
                        st_eng.dma_start(
                            out_v[ct][:, b, s * SR2 * W:(s + 1) * SR2 * W],
                            fin[:])

            # ===== schedule =====
            # b0's gate + postprocessing are emitted BETWEEN conv(0) and
            # conv(1) so they sit ahead of conv-b1 in the per-engine FIFOs
            # and execute during conv-b1's matmuls.
            if phase_limit >= 1:
                phase_a1(0)
                phase_a1(1)
            if phase_limit >= 2:
                phase_conv(0)
                phase_conv(1, 0, 1)
            if phase_limit >= 3:
                se_gate(0, 0, w1a, w2a, b1a, V_B2A, V_INV2, 8)
            if phase_limit >= 2:
                phase_conv(1, 1, NCH)
            if phase_limit >= 4:
                phase_a4(0)
            if phase_limit >= 3:
                se_gate(1, 0, w1a, w2a, b1a, V_B2A, V_INV2, 8)
            if phase_limit >= 5:
                phase_f1(0)
            if phase_limit >= 6:
                se_gate(0, 1, w1f, w2f, b1f, V_B2F, V_FINV2, 10,
                        sum_corr_col=V_SB2HW)
            if phase_limit >= 4:
                phase_a4(1)
            if phase_limit >= 7:
                phase_f3(0)
            if phase_limit >= 5:
                phase_f1(1)
            if phase_limit >= 6:
                se_gate(1, 1, w1f, w2f, b1f, V_B2F, V_FINV2, 10,
                        sum_corr_col=V_SB2HW)
            if phase_limit >= 7:
                phase_f3(1)

    nc.compile()
    return nc


def _host_prep(inputs):
    import ml_dtypes
    f8 = ml_dtypes.float8_e4m3
    f32 = np.float32
    g1, be1, m1, v1 = (inputs["a_bn1"][i].astype(f32) for i in range(4))
    g2, be2, m2, v2 = (inputs["a_bn2"][i].astype(f32) for i in range(4))
    fg1, fbe1, fm1, fv1 = (inputs["f_bn1"][i].astype(f32) for i in range(4))
    fg2, fbe2, fm2, fv2 = (inputs["f_bn2"][i].astype(f32) for i in range(4))
    inv1 = g1 / np.sqrt(v1 + EPS)
    bias1 = be1 - m1 * inv1
    inv2 = g2 / np.sqrt(v2 + EPS)
    bias2 = be2 - m2 * inv2
    finv1 = fg1 / np.sqrt(fv1 + EPS)
    fbias1 = fbe1 - fm1 * finv1
    finv2 = fg2 / np.sqrt(fv2 + EPS)
    fbias2 = fbe2 - fm2 * finv2

    s = inputs["a_scale"].astype(f32)
    fs = inputs["f_scale"].astype(f32)

    # fp8 per-output-channel pow2 scaling (folded into the bn1 drain scale);
    # z is stored as +-0.5 so the drain scale carries an extra 2x
    bw = np.clip(inputs["a_w"].astype(f32), -1.0, 1.0)        # [O, I, 3, 3]
    mxa = np.abs(bw.reshape(C, -1)).max(axis=1)
    sa = np.exp2(np.floor(np.log2(224.0 / np.where(mxa == 0, 1.0, mxa))))
    bw2 = np.clip(inputs["f_w"].astype(f32), -1.0, 1.0)       # [O, I]
    mxf = np.abs(bw2).max(axis=1)
    sf = np.exp2(np.floor(np.log2(224.0 / np.where(mxf == 0, 1.0, mxf))))

    vecs = np.zeros((C, NV), f32)
    vecs[:, V_THR1] = -inputs["a_move"]
    vecs[:, 1] = inputs["a_alpha1"]
    vecs[:, 2] = 2.0 * inv1 / sa
    vecs[:, 3] = bias1
    vecs[:, 4] = inputs["a_alpha2"]
    vecs[:, 5] = s
    vecs[:, 6] = 1.0 - s
    vecs[:, 7] = inv2
    vecs[:, V_THR2] = -(bias2 + inputs["f_move"])
    vecs[:, 9] = 2.0 * finv1 / sf
    vecs[:, 10] = fbias1
    vecs[:, 11] = inputs["f_alpha2"]
    vecs[:, 12] = fs
    vecs[:, 13] = 1.0 - fs
    vecs[:, 14] = finv2
    vecs[:, 15] = fbias2 + bias2
    vecs[:, 16] = inputs["a_se_b2"]
    vecs[:, 17] = inputs["f_se_b2"]
    vecs[:, 18] = float(HW) * bias2
    vecs[:, 19] = bias2
    vecs_ct = np.ascontiguousarray(vecs.reshape(CT, 128, NV))

    # wa[t, p, kt, o] = bw[o, kt*128+p, ky, kx] * sa[o]
    bws = bw * sa[:, None, None, None]
    bwT = np.transpose(bws, (1, 0, 2, 3)).reshape(CT, 128, C, 3, 3)
    wa_h = np.zeros((9, 128, CT, C), f32)
    for ky in range(3):
        for kx in range(3):
            wa_h[ky * 3 + kx] = np.transpose(bwT[:, :, :, ky, kx], (1, 0, 2))
    wa_h = wa_h.astype(f8)
    bw2s = (bw2 * sf[:, None]).T.reshape(CT, 128, C)
    wfm = np.ascontiguousarray(np.transpose(bw2s, (1, 0, 2))).astype(f8)

    def w1_fold(w1, sv):
        w1t = w1.astype(f32).T / float(HW)          # [256, 32]
        out = np.zeros((2, CT, 128, 32), f32)
        out[0] = (w1t * sv[:, None]).reshape(CT, 128, 32)
        out[1] = (w1t * (1.0 - sv)[:, None]).reshape(CT, 128, 32)
        return out

    common = {
        "wa": wa_h, "wf": wfm, "vecs": vecs_ct,
        "b1a": inputs["a_se_b1"].astype(f32).reshape(32, 1),
        "b1f": inputs["f_se_b1"].astype(f32).reshape(32, 1),
        "w1a": w1_fold(inputs["a_se_w1"], s),
        "w2a": np.ascontiguousarray(inputs["a_se_w2"].astype(f32).T),
        "w1f": w1_fold(inputs["f_se_w1"], fs),
        "w2f": np.ascontiguousarray(inputs["f_se_w2"].astype(f32).T),
    }
    return common


V_THR1 = 0
V_THR2 = 8


def kernel(**inputs):
    from concourse import bass_utils

    if "nc" not in _CACHE:
        _CACHE["nc"] = _build_program()
    nc = _CACHE["nc"]

    import ml_dtypes
    common = _host_prep(inputs)
    x = np.ascontiguousarray(
        inputs["x"].astype(np.float32).astype(ml_dtypes.bfloat16))
    in_maps = []
    for c in range(NCORES):
        m = dict(common)
        m["xin"] = np.ascontiguousarray(x[c * BP:(c + 1) * BP])
        in_maps.append(m)

    res = None
    for attempt in range(3):
        try:
            res = bass_utils.run_bass_kernel_spmd(
                nc, in_maps, core_ids=list(range(NCORES)))
            break
        except Exception:
            # transient device wedge on a freshly loaded NEFF: retry
            if attempt == 2:
                raise
    out = np.empty((B, C, H, W), np.float32)
    for c in range(NCORES):
        out[c * BP:(c + 1) * BP] = res.results[c]["out"].astype(np.float32)
    return out
